# revision 6
# baseline (speedup 1.0000x reference)
"""Trainium2 Bass kernel for nn_Block_9268539425531 (MLA transformer block).

Sharding: 2 batch groups x 4-way TP within each group of 4 cores.
Per core (b = core//4, r = core%4, heads H = [4r, 4r+4)):
  Phase A: ln1 + w_down on own token slice (512 tokens), AllGather h+dkv.
  Phase B: q/k/v/qR/kR projections for own 4 heads, all 2048 tokens,
           spilled to DRAM.
  Phase C: causal attention for own 4 heads (scoresT layout, matmul-based
           partition softmax reductions), AllGather oT.
  Phase D: w_o + residual + ln2 on own token slice.
  Phase E: FFN (full hidden dim, own token slice) + residual.
All matmuls in float32r (full-rate fp32, ~1.6e-4 rel rounding).
"""
import math
import numpy as np

B, T, C = 2, 2048, 2048
NH = 16
DK = 128
DHR = 64
LAT = 512
P = 128
NT = 512           # tokens per core
CC = C // P        # 16
NCORES = 8
SCALE = 1.0 / math.sqrt(DK)
NEG = -1.0e9
RG = [[0, 1, 2, 3], [4, 5, 6, 7]]

_CACHE = {}


# ---------------------------------------------------------------- program ---
def build_program(repeat=1):
    from contextlib import ExitStack
    from concourse import bass, bacc, tile, mybir

    dt = mybir.dt
    f32 = dt.float32
    f32r = dt.float32r
    AF = mybir.ActivationFunctionType
    OP = mybir.AluOpType

    nc = bacc.Bacc("TRN2", target_bir_lowering=False, debug=False,
                   num_devices=NCORES)

    def din(name, shape, dtype=f32r):
        return nc.dram_tensor(name, shape, dtype, kind="ExternalInput")

    xT_d = din("xT", [CC, P, NT], f32)
    ln1s_d = din("ln1s", [P, CC], f32)
    ln1b_d = din("ln1b", [P, CC], f32)
    ln2s_d = din("ln2s", [P, CC], f32)
    ln2b_d = din("ln2b", [P, CC], f32)
    wdown_d = din("wdown", [CC, P, 8 * P])
    bdown_d = din("bdown", [P, 8], f32)
    wqr_d = din("wqr", [CC, P, 2 * P])
    bqr_d = din("bqr", [P, 2], f32)
    wkr_d = din("wkr", [CC, P, P])
    bkr_d = din("bkr", [P, 1], f32)
    r2_d = din("r2", [P, P])
    cosq_d = din("cosq", [2, 4, P, NT], f32)
    sinq_d = din("sinq", [2, 4, P, NT], f32)
    cosk_d = din("cosk", [4, P, NT], f32)
    sink_d = din("sink", [4, P, NT], f32)
    wuk_d = din("wuk", [4, P, 4 * P])
    buk_d = din("buk", [P, 4], f32)
    wuv_d = din("wuv", [4, P, 4 * P])
    buv_d = din("buv", [P, 4], f32)
    wuq_d = din("wuq", [4, P, 4 * P])
    buq_d = din("buq", [P, 4], f32)
    mask_d = din("mask", [4, P, NT], f32)
    ones_r_d = din("ones_r", [P, P])
    ones32_d = din("ones32", [P, P], f32)
    wo_d = din("wo", [CC, 4, P, 4 * P])
    bo_d = din("bo", [P, CC], f32)
    wff1_d = din("wff1", [CC, 16, P, 4 * P])
    bff1_d = din("bff1", [P, 64], f32)
    wff2_d = din("wff2", [4, CC, P, CC * P])
    bff2_d = din("bff2", [P, CC], f32)
    outT_d = nc.dram_tensor("outT", [CC, P, NT], f32, kind="ExternalOutput")

    with tile.TileContext(nc) as tc, ExitStack() as ctx:
        pc = ctx.enter_context(tc.tile_pool(name="const", bufs=1))
        pdram = ctx.enter_context(tc.tile_pool(name="dram", bufs=1, space="DRAM"))

        # ---- small constants resident for the whole kernel (~3KB/part)
        ones_r = pc.tile([P, P], f32r)
        nc.sync.dma_start(ones_r[:], ones_r_d[:])
        ones32 = pc.tile([P, P], f32)
        nc.sync.dma_start(ones32[:], ones32_d[:])
        r2 = pc.tile([P, P], f32r)
        nc.sync.dma_start(r2[:], r2_d[:])
        ln1s = pc.tile([P, CC], f32)
        nc.sync.dma_start(ln1s[:], ln1s_d[:])
        ln1b = pc.tile([P, CC], f32)
        nc.sync.dma_start(ln1b[:], ln1b_d[:])
        ln2s = pc.tile([P, CC], f32)
        nc.sync.dma_start(ln2s[:], ln2s_d[:])
        ln2b = pc.tile([P, CC], f32)
        nc.sync.dma_start(ln2b[:], ln2b_d[:])
        bdown = pc.tile([P, 8], f32)
        nc.sync.dma_start(bdown[:], bdown_d[:])
        bqr = pc.tile([P, 2], f32)
        nc.sync.dma_start(bqr[:], bqr_d[:])
        bkr = pc.tile([P, 1], f32)
        nc.sync.dma_start(bkr[:], bkr_d[:])
        buk = pc.tile([P, 4], f32)
        nc.sync.dma_start(buk[:], buk_d[:])
        buv = pc.tile([P, 4], f32)
        nc.sync.dma_start(buv[:], buv_d[:])
        buq = pc.tile([P, 4], f32)
        nc.sync.dma_start(buq[:], buq_d[:])
        bo = pc.tile([P, CC], f32)
        nc.sync.dma_start(bo[:], bo_d[:])
        bff1 = pc.tile([P, 64], f32)
        nc.sync.dma_start(bff1[:], bff1_d[:])
        bff2 = pc.tile([P, CC], f32)
        nc.sync.dma_start(bff2[:], bff2_d[:])
        eps_t = pc.tile([P, 1], f32)
        nc.vector.memset(eps_t[:], 1e-6)

        agin1 = pdram.tile([3072, NT], f32r)          # h (2048) + dkv (1024)
        agout1 = pdram.tile([4 * 3072, NT], f32r)
        agin2 = pdram.tile([4 * P, T], f32r)          # own-heads oT
        agout2 = pdram.tile([16 * P, T], f32r)
        qR_sp = pdram.tile([2, P, T], f32r)           # projection spills
        kR_sp = pdram.tile([P, T], f32r)
        q_sp = pdram.tile([4, P, T], f32r)
        k_sp = pdram.tile([4, P, T], f32r)
        v_sp = pdram.tile([16, P, 4 * P], f32r)

        pid = nc.sync.partition_id()
        colo = (pid % 4) * NT

        def layer_norm(src_tiles, pstream, pstat, pool_ps, lns, lnb,
                       out_pool, out_name, rep):
            """src [16][P, NT] fp32 -> normalized f32r tiles (list)."""
            ps_mean = pool_ps.tile([P, NT], f32, name=f"lnpm{rep}{out_name}")
            ps_sq = pool_ps.tile([P, NT], f32, name=f"lnps{rep}{out_name}")
            for ci in range(CC):
                sq = pstream.tile([P, NT], f32r, name="lnsq", tag="lnsq")
                nc.scalar.square(sq[:], src_tiles[ci][:])
                nc.tensor.matmul(ps_mean[:], ones32[:], src_tiles[ci][:],
                                 start=(ci == 0), stop=(ci == CC - 1),
                                 skip_group_check=True)
                nc.tensor.matmul(ps_sq[:], ones_r[:], sq[:],
                                 start=(ci == 0), stop=(ci == CC - 1),
                                 skip_group_check=True)
            meanb = pstat.tile([P, NT], f32, name="lnmean", tag="lnmean")
            nc.vector.tensor_scalar_mul(meanb[:], ps_mean[:], 1.0 / C)
            m2 = pstat.tile([P, NT], f32, name="lnm2", tag="lnm2")
            nc.vector.tensor_mul(m2[:], meanb[:], meanb[:])
            var = pstat.tile([P, NT], f32, name="lnvar", tag="lnvar")
            nc.vector.scalar_tensor_tensor(var[:], ps_sq[:], 1.0 / C, m2[:],
                                           OP.mult, OP.subtract)
            std = pstat.tile([P, NT], f32, name="lnstd", tag="lnstd")
            nc.scalar.activation(std[:], var[:], AF.Sqrt, bias=eps_t[:])
            rstd = pstat.tile([P, NT], f32, name="lnrstd", tag="lnrstd")
            nc.vector.reciprocal(rstd[:], std[:])
            outs = []
            for ci in range(CC):
                t1 = pstream.tile([P, NT], f32, name="lnt1", tag="lnt1")
                nc.vector.tensor_sub(t1[:], src_tiles[ci][:], meanb[:])
                t2 = pstream.tile([P, NT], f32, name="lnt2", tag="lnt2")
                nc.vector.tensor_mul(t2[:], t1[:], rstd[:])
                h = out_pool.tile([P, NT], f32r, name=f"{out_name}{ci}")
                nc.vector.tensor_scalar(h[:], t2[:], lns[:, ci:ci + 1],
                                        lnb[:, ci:ci + 1], OP.mult, OP.add)
                outs.append(h)
            return outs

        for rep in range(repeat):
            # ------------------------------------------------ phase A ----
            with (tc.tile_pool(name=f"pxa{rep}", bufs=1) as pxa,
                  tc.tile_pool(name=f"pa{rep}", bufs=3) as pa,
                  tc.tile_pool(name=f"pas{rep}", bufs=1) as pas,
                  tc.tile_pool(name=f"pah{rep}", bufs=1) as pah,
                  tc.tile_pool(name=f"paw{rep}", bufs=20) as paw,
                  tc.tile_pool(name=f"paps{rep}", bufs=2, space="PSUM") as paps,
                  tc.tile_pool(name=f"past{rep}", bufs=1, space="PSUM") as pstat):
                xT = []
                for ci in range(CC):
                    t = pxa.tile([P, NT], f32, name=f"xTa{ci}")
                    nc.sync.dma_start(t[:], xT_d[ci])
                    xT.append(t)
                hts = layer_norm(xT, pa, pas, pstat, ln1s, ln1b, pah, "h", rep)
                for ci in range(CC):
                    nc.sync.dma_start(agin1[P * ci:P * (ci + 1), :], hts[ci][:])
                for mi in range(8):
                    ps = paps.tile([P, NT], f32, name="psdkv", tag="psdkv")
                    for ci in range(CC):
                        w = paw.tile([P, P], f32r, name="wdt", tag="wdt")
                        nc.sync.dma_start(w[:], wdown_d[ci, :, mi * P:(mi + 1) * P])
                        nc.tensor.matmul(ps[:], w[:], hts[ci][:],
                                         start=(ci == 0), stop=(ci == CC - 1))
                    dkv = pa.tile([P, NT], f32r, name="dkvt", tag="dkvt")
                    nc.vector.tensor_scalar_add(dkv[:], ps[:], bdown[:, mi:mi + 1])
                    nc.sync.dma_start(agin1[C + P * mi:C + P * (mi + 1), :], dkv[:])

            nc.gpsimd.collective_compute(
                "AllGather", mybir.AluOpType.bypass, replica_groups=RG,
                ins=[agin1.opt()], outs=[agout1.opt()])

            # --------------------------------------------- phase B1 ----
            # qR, kR, q projections -> DRAM spills
            with (tc.tile_pool(name=f"pb1w{rep}", bufs=1) as pw,
                  tc.tile_pool(name=f"pb1s{rep}", bufs=18) as pstream,
                  tc.tile_pool(name=f"pb1c{rep}", bufs=6) as pcq,
                  tc.tile_pool(name=f"pb1t{rep}", bufs=3) as pt,
                  tc.tile_pool(name=f"pb1cs{rep}", bufs=3) as pcs,
                  tc.tile_pool(name=f"pb1ps{rep}", bufs=3, space="PSUM") as pps,
                  tc.tile_pool(name=f"pb1pr{rep}", bufs=2, space="PSUM") as ppsr):
                wqr_sb = []
                for ci in range(CC):
                    w = pw.tile([P, 2 * P], f32r, name=f"wqr{ci}")
                    nc.sync.dma_start(w[:], wqr_d[ci])
                    wqr_sb.append(w)
                wkr_sb = []
                for ci in range(CC):
                    w = pw.tile([P, P], f32r, name=f"wkr{ci}")
                    nc.sync.dma_start(w[:], wkr_d[ci])
                    wkr_sb.append(w)
                wuq_sb = []
                for lc in range(4):
                    w = pw.tile([P, 4 * P], f32r, name=f"wuq{lc}")
                    nc.sync.dma_start(w[:], wuq_d[lc])
                    wuq_sb.append(w)

                def rope(pre, cos_t, sin_t, dst_ap):
                    rot = ppsr.tile([P, NT], f32, name="psrot", tag="psrot")
                    nc.tensor.matmul(rot[:], r2[:], pre[:], start=True, stop=True)
                    tmp = pt.tile([P, NT], f32, name="rtmp", tag="rtmp")
                    nc.vector.tensor_mul(tmp[:], rot[:], sin_t[:])
                    tmp2 = pt.tile([P, NT], f32, name="rtmp2", tag="rtmp2")
                    nc.vector.tensor_mul(tmp2[:], pre[:], cos_t[:])
                    out = pt.tile([P, NT], f32r, name="rout", tag="rout")
                    nc.vector.tensor_add(out[:], tmp2[:], tmp[:])
                    nc.sync.dma_start(dst_ap, out[:])

                for nt in range(4):
                    nts = slice(nt * NT, (nt + 1) * NT)
                    ht = []
                    for ci in range(CC):
                        t = pstream.tile([P, NT], f32r, name="htb", tag="htb")
                        nc.sync.dma_start(
                            t[:], agout1[3072 * nt + P * ci:
                                         3072 * nt + P * (ci + 1), :])
                        ht.append(t)
                    cq = []
                    for lc in range(4):
                        t = pcq.tile([P, NT], f32r, name="cqb", tag="cqb")
                        nc.sync.dma_start(
                            t[:], agout1[3072 * nt + 2560 + P * lc:
                                         3072 * nt + 2560 + P * (lc + 1), :])
                        cq.append(t)
                    for mt in range(2):
                        ps = pps.tile([P, NT], f32, name="psqr", tag="psqr")
                        for ci in range(CC):
                            nc.tensor.matmul(
                                ps[:], wqr_sb[ci][:, mt * P:(mt + 1) * P],
                                ht[ci][:], start=(ci == 0), stop=(ci == CC - 1))
                        pre = pt.tile([P, NT], f32r, name="qrpre", tag="qrpre")
                        nc.scalar.activation(pre[:], ps[:], AF.Identity,
                                             bias=bqr[:, mt:mt + 1])
                        cos_t = pcs.tile([P, NT], f32, name="cosq", tag="cosq")
                        nc.sync.dma_start(cos_t[:], cosq_d[mt, nt])
                        sin_t = pcs.tile([P, NT], f32, name="sinq", tag="sinq")
                        nc.sync.dma_start(sin_t[:], sinq_d[mt, nt])
                        rope(pre, cos_t, sin_t, qR_sp[mt][:, nts])
                    # kR
                    ps = pps.tile([P, NT], f32, name="pskr", tag="psqr")
                    for ci in range(CC):
                        nc.tensor.matmul(ps[:], wkr_sb[ci][:], ht[ci][:],
                                         start=(ci == 0), stop=(ci == CC - 1))
                    pre = pt.tile([P, NT], f32r, name="krpre", tag="qrpre")
                    nc.scalar.activation(pre[:], ps[:], AF.Identity,
                                         bias=bkr[:, 0:1])
                    cos_t = pcs.tile([P, NT], f32, name="cosk", tag="cosq")
                    nc.sync.dma_start(cos_t[:], cosk_d[nt])
                    sin_t = pcs.tile([P, NT], f32, name="sink", tag="sinq")
                    nc.sync.dma_start(sin_t[:], sink_d[nt])
                    rope(pre, cos_t, sin_t, kR_sp[:, nts])
                    # q
                    for mt in range(4):
                        ps = pps.tile([P, NT], f32, name="psq", tag="psqr")
                        for lc in range(4):
                            nc.tensor.matmul(
                                ps[:], wuq_sb[lc][:, mt * P:(mt + 1) * P],
                                cq[lc][:], start=(lc == 0), stop=(lc == 3))
                        qo = pt.tile([P, NT], f32r, name="qout", tag="rout")
                        nc.vector.tensor_scalar_add(qo[:], ps[:],
                                                    buq[:, mt:mt + 1])
                        nc.sync.dma_start(q_sp[mt][:, nts], qo[:])

            # --------------------------------------------- phase B2 ----
            with (tc.tile_pool(name=f"pb2w{rep}", bufs=1) as pw2,
                  tc.tile_pool(name=f"pb2s{rep}", bufs=6) as pkv,
                  tc.tile_pool(name=f"pb2t{rep}", bufs=3) as pt2,
                  tc.tile_pool(name=f"pb2ps{rep}", bufs=3, space="PSUM") as pps2):
                wuk_sb = []
                wuv_sb = []
                for lc in range(4):
                    w = pw2.tile([P, 4 * P], f32r, name=f"wuk{lc}")
                    nc.sync.dma_start(w[:], wuk_d[lc])
                    wuk_sb.append(w)
                    w = pw2.tile([P, 4 * P], f32r, name=f"wuv{lc}")
                    nc.sync.dma_start(w[:], wuv_d[lc])
                    wuv_sb.append(w)
                for nt in range(4):
                    nts = slice(nt * NT, (nt + 1) * NT)
                    ckv = []
                    for lc in range(4):
                        t = pkv.tile([P, NT], f32r, name="ckvb", tag="ckvb")
                        nc.sync.dma_start(
                            t[:], agout1[3072 * nt + 2048 + P * lc:
                                         3072 * nt + 2048 + P * (lc + 1), :])
                        ckv.append(t)
                    for mt in range(4):
                        ps = pps2.tile([P, NT], f32, name="psk", tag="psk")
                        for lc in range(4):
                            nc.tensor.matmul(
                                ps[:], wuk_sb[lc][:, mt * P:(mt + 1) * P],
                                ckv[lc][:], start=(lc == 0), stop=(lc == 3))
                        ko = pt2.tile([P, NT], f32r, name="kout", tag="kout")
                        nc.vector.tensor_scalar_add(ko[:], ps[:],
                                                    buk[:, mt:mt + 1])
                        nc.sync.dma_start(k_sp[mt][:, nts], ko[:])
                    for tt in range(4):
                        ps = pps2.tile([P, 4 * P], f32, name="psv", tag="psk")
                        for lc in range(4):
                            nc.tensor.matmul(
                                ps[:], ckv[lc][:, tt * P:(tt + 1) * P],
                                wuv_sb[lc][:], start=(lc == 0), stop=(lc == 3))
                        vo = pt2.tile([P, 4 * P], f32r, name="vout", tag="vout")
                        nc.vector.tensor_copy(vo[:], ps[:])
                        nc.sync.dma_start(v_sp[4 * nt + tt], vo[:])

            # ---------------------------------------------- phase C ----
            with (tc.tile_pool(name=f"pch{rep}", bufs=2) as phd,
                  tc.tile_pool(name=f"pcm{rep}", bufs=1) as pcm,
                  tc.tile_pool(name=f"pce{rep}", bufs=4) as pex,
                  tc.tile_pool(name=f"pco{rep}", bufs=3) as pot,
                  tc.tile_pool(name=f"pcps{rep}", bufs=3, space="PSUM") as pcsc,
                  tc.tile_pool(name=f"pcpo{rep}", bufs=2, space="PSUM") as pcso,
                  tc.tile_pool(name=f"pcpm{rep}", bufs=2, space="PSUM") as pcss):
                masks = []
                for j in range(4):
                    m_ = pcm.tile([P, NT], f32, name=f"mask{j}")
                    nc.sync.dma_start(m_[:], mask_d[j])
                    masks.append(m_)
                for h in range(4):
                    kTh = phd.tile([P, T], f32r, name="kTh", tag="kTh")
                    nc.sync.dma_start(kTh[:], k_sp[h])
                    qTh = phd.tile([P, T], f32r, name="qTh", tag="qTh")
                    nc.sync.dma_start(qTh[:], q_sp[h])
                    qRh = phd.tile([DHR, T], f32r, name="qRh", tag="qRh")
                    nc.sync.dma_start(
                        qRh[:], qR_sp[h // 2][DHR * (h % 2):DHR * (h % 2) + DHR, :])
                    kRh = phd.tile([DHR, T], f32r, name="kRh", tag="kRh")
                    nc.sync.dma_start(kRh[:], kR_sp[0:DHR, :])
                    vh = phd.tile([P, 16, P], f32r, name="vh", tag="vh")
                    for tt in range(16):
                        nc.sync.dma_start(vh[:, tt, :],
                                          v_sp[tt][:, h * P:(h + 1) * P])
                    for qi in range(4):
                        qs = slice(qi * NT, (qi + 1) * NT)
                        pso = pcso.tile([P, NT], f32, name="pso", tag="pso")
                        pss = pcss.tile([P, NT], f32, name="pss", tag="pss")
                        nki = 4 * qi + 4
                        for ki in range(nki):
                            ks = slice(ki * P, (ki + 1) * P)
                            psc = pcsc.tile([P, NT], f32, name="psc", tag="psc")
                            nc.tensor.matmul(psc[:], kTh[:, ks], qTh[:, qs],
                                             start=True, stop=False)
                            nc.tensor.matmul(psc[:], kRh[:, ks], qRh[:, qs],
                                             start=False, stop=True)
                            if ki >= 4 * qi:
                                nc.vector.tensor_add(psc[:], psc[:],
                                                     masks[ki - 4 * qi][:])
                            ex = pex.tile([P, NT], f32r, name="ex", tag="ex")
                            nc.scalar.activation(ex[:], psc[:], AF.Exp,
                                                 scale=SCALE)
                            nc.tensor.matmul(pso[:], vh[:, ki, :], ex[:],
                                             start=(ki == 0),
                                             stop=(ki == nki - 1))
                            nc.tensor.matmul(pss[:], ones_r[:], ex[:],
                                             start=(ki == 0),
                                             stop=(ki == nki - 1))
                        rec = pot.tile([P, NT], f32, name="rec", tag="rec")
                        nc.vector.reciprocal(rec[:], pss[:])
                        ot = pot.tile([P, NT], f32, name="ot", tag="ot")
                        nc.vector.tensor_mul(ot[:], pso[:], rec[:])
                        otb = pot.tile([P, NT], f32r, name="otb", tag="otb")
                        nc.vector.tensor_scalar_add(otb[:], ot[:],
                                                    buv[:, h:h + 1])
                        nc.sync.dma_start(agin2[h * P:(h + 1) * P, qs], otb[:])

            nc.gpsimd.collective_compute(
                "AllGather", mybir.AluOpType.bypass, replica_groups=RG,
                ins=[agin2.opt()], outs=[agout2.opt()])

            # ------------------------------------------------ phase D ----
            with tc.tile_pool(name=f"pde{rep}", bufs=1) as pper:
                with (tc.tile_pool(name=f"pxd{rep}", bufs=1) as pxd,
                      tc.tile_pool(name=f"pdo{rep}", bufs=1) as pdo,
                      tc.tile_pool(name=f"pdw{rep}", bufs=17) as pdw,
                      tc.tile_pool(name=f"pdt{rep}", bufs=3) as pdt,
                      tc.tile_pool(name=f"pds{rep}", bufs=1) as pds,
                      tc.tile_pool(name=f"pdps{rep}", bufs=3, space="PSUM") as pdps,
                      tc.tile_pool(name=f"pdst{rep}", bufs=1, space="PSUM") as pdst):
                    xT2 = []
                    for ci in range(CC):
                        t = pxd.tile([P, NT], f32, name=f"xTd{ci}")
                        nc.sync.dma_start(t[:], xT_d[ci])
                        xT2.append(t)
                    otsl = []
                    for oi in range(16):
                        t = pdo.tile([P, NT], f32r, name=f"otsl{oi}")
                        nc.sync.dma_start(
                            t[:], agout2[oi * P:(oi + 1) * P, bass.ds(colo, NT)])
                        otsl.append(t)
                    xmid = []
                    for mig in range(4):
                        wots = []
                        for ki in range(16):
                            w = pdw.tile([P, 4 * P], f32r, name="wot", tag="wot")
                            nc.sync.dma_start(w[:], wo_d[ki, mig])
                            wots.append(w)
                        for ml in range(4):
                            mi = mig * 4 + ml
                            ps = pdps.tile([P, NT], f32, name="pswo", tag="pswo")
                            for ki in range(16):
                                nc.tensor.matmul(
                                    ps[:], wots[ki][:, ml * P:(ml + 1) * P],
                                    otsl[ki][:], start=(ki == 0), stop=(ki == 15))
                            xm = pper.tile([P, NT], f32, name=f"xmid{mi}")
                            nc.vector.scalar_tensor_tensor(
                                xm[:], ps[:], bo[:, mi:mi + 1], xT2[mi][:],
                                OP.add, OP.add)
                            xmid.append(xm)
                    h2 = layer_norm(xmid, pdt, pds, pdst, ln2s, ln2b, pper,
                                    "h2_", rep)

                # -------------------------------------------- phase E ----
                with (tc.tile_pool(name=f"pew{rep}", bufs=17) as pew,
                      tc.tile_pool(name=f"pew2{rep}", bufs=2) as pew2,
                      tc.tile_pool(name=f"peg{rep}", bufs=17) as peg,
                      tc.tile_pool(name=f"pea{rep}", bufs=1) as pea,
                      tc.tile_pool(name=f"pet{rep}", bufs=3) as pet,
                      tc.tile_pool(name=f"peps{rep}", bufs=3, space="PSUM") as peps,
                      tc.tile_pool(name=f"pep2{rep}", bufs=2, space="PSUM") as pep2):
                    accs = [pea.tile([P, NT], f32, name=f"ffacc{mi}")
                            for mi in range(16)]
                    for hb in range(4):
                        gts = []
                        for mtg in range(4):
                            mtg_g = hb * 4 + mtg
                            wts = []
                            for ci in range(CC):
                                w = pew.tile([P, 4 * P], f32r, name="wf1",
                                             tag="wf1")
                                nc.sync.dma_start(w[:], wff1_d[ci, mtg_g])
                                wts.append(w)
                            for ml in range(4):
                                mt = mtg_g * 4 + ml
                                ps = peps.tile([P, NT], f32, name="psf1",
                                               tag="psf1")
                                for ci in range(CC):
                                    nc.tensor.matmul(
                                        ps[:], wts[ci][:, ml * P:(ml + 1) * P],
                                        h2[ci][:],
                                        start=(ci == 0), stop=(ci == CC - 1))
                                gt = peg.tile([P, NT], f32r, name="gt", tag="gt")
                                nc.scalar.activation(gt[:], ps[:],
                                                     AF.Gelu_apprx_tanh,
                                                     bias=bff1[:, mt:mt + 1])
                                gts.append(gt)
                        for mi in range(16):
                            w2 = pew2.tile([P, CC * P], f32r, name="wf2",
                                           tag="wf2")
                            nc.sync.dma_start(w2[:], wff2_d[hb, mi])
                            ps2 = pep2.tile([P, NT], f32, name="psf2",
                                            tag="psf2")
                            for hl in range(16):
                                nc.tensor.matmul(
                                    ps2[:], w2[:, hl * P:(hl + 1) * P],
                                    gts[hl][:],
                                    start=(hl == 0), stop=(hl == 15))
                            if hb == 0:
                                nc.vector.tensor_copy(accs[mi][:], ps2[:])
                            else:
                                nc.vector.tensor_add(accs[mi][:], accs[mi][:],
                                                     ps2[:])
                    for mi in range(CC):
                        ob = pet.tile([P, NT], f32, name="outb", tag="outb")
                        nc.vector.scalar_tensor_tensor(
                            ob[:], accs[mi][:], bff2[:, mi:mi + 1],
                            xmid[mi][:], OP.add, OP.add)
                        nc.sync.dma_start(outT_d[mi], ob[:])

    nc.compile()
    return nc


# ------------------------------------------------------------------ host ---
def _rope_tables(r):
    """cos/sin tiles for core rank r (heads 4r..4r+3)."""
    t = np.arange(T, dtype=np.float64) + 1.0
    l = np.arange(DHR)
    cosq = np.zeros((2, P, T), np.float64)
    sinq = np.zeros((2, P, T), np.float64)
    for mt in range(2):
        for hl in range(2):
            h = 4 * r + 2 * mt + hl
            theta = 10000.0 ** (-2.0 * (32 * h + l // 2) / 1024.0)
            ang = t[None, :] * theta[:, None]            # [64, T]
            cosq[mt, 64 * hl:64 * hl + 64] = np.cos(ang)
            sinq[mt, 64 * hl:64 * hl + 64] = np.sin(ang)
    thk = 10000.0 ** (-2.0 * (l // 2) / 64.0)
    angk = t[None, :] * thk[:, None]
    cosk = np.concatenate([np.cos(angk)] * 2, axis=0)     # [128, T]
    sink = np.concatenate([np.sin(angk)] * 2, axis=0)
    cosq = cosq.reshape(2, P, 4, NT).transpose(0, 2, 1, 3)
    sinq = sinq.reshape(2, P, 4, NT).transpose(0, 2, 1, 3)
    cosk = cosk.reshape(P, 4, NT).transpose(1, 0, 2)
    sink = sink.reshape(P, 4, NT).transpose(1, 0, 2)
    f = np.float32
    return (np.ascontiguousarray(cosq, f), np.ascontiguousarray(sinq, f),
            np.ascontiguousarray(cosk, f), np.ascontiguousarray(sink, f))


def _shared_consts():
    r2 = np.zeros((P, P), np.float32)
    for i in range(64):
        r2[2 * i + 1, 2 * i] = -1.0
        r2[2 * i, 2 * i + 1] = 1.0
    mask = np.zeros((4, P, NT), np.float32)
    kl = np.arange(P)[:, None]
    ql = np.arange(NT)[None, :]
    for j in range(4):
        mask[j] = np.where(P * j + kl > ql, NEG, 0.0)
    ones = np.ones((P, P), np.float32)
    return r2, mask, ones


def prepare_in_maps(inputs):
    f = np.float32
    g = {k: np.asarray(v, f) for k, v in inputs.items()}
    x = g["x"]
    r2, mask, ones = _shared_consts()

    wdown_t = np.ascontiguousarray(g["w_down"].reshape(CC, P, 8 * P))
    bdown_t = np.ascontiguousarray(g["b_down"].reshape(8, P).T)
    wkr2 = np.concatenate([g["w_kr"], g["w_kr"]], axis=1)  # [C, 128]
    wkr_t = np.ascontiguousarray(wkr2.reshape(CC, P, P))
    bkr_t = np.ascontiguousarray(
        np.concatenate([g["b_kr"], g["b_kr"]]).reshape(P, 1))
    wo_t = np.ascontiguousarray(
        g["w_o"].reshape(CC, P, 4, 4 * P).transpose(0, 2, 1, 3))
    bo_t = np.ascontiguousarray(g["b_o"].reshape(CC, P).T)
    wff1_t = np.ascontiguousarray(
        g["w_ff1"].reshape(CC, P, 16, 4 * P).transpose(0, 2, 1, 3))
    bff1_t = np.ascontiguousarray(g["b_ff1"].reshape(64, P).T)
    wff2_t = np.ascontiguousarray(
        g["w_ff2"].reshape(4, CC, P, CC, P).transpose(0, 3, 2, 1, 4)
        .reshape(4, CC, P, CC * P))
    bff2_t = np.ascontiguousarray(g["b_ff2"].reshape(CC, P).T)
    ln1s_t = np.ascontiguousarray(g["ln1_scale"].reshape(CC, P).T)
    ln1b_t = np.ascontiguousarray(g["ln1_bias"].reshape(CC, P).T)
    ln2s_t = np.ascontiguousarray(g["ln2_scale"].reshape(CC, P).T)
    ln2b_t = np.ascontiguousarray(g["ln2_bias"].reshape(CC, P).T)

    in_maps = []
    for c in range(NCORES):
        b, r = divmod(c, 4)
        cosq, sinq, cosk, sink = _rope_tables(r)
        xs = x[b, NT * r:NT * (r + 1), :].T                      # [C, NT]
        xT_t = np.ascontiguousarray(xs.reshape(CC, P, NT))
        wqr_c = g["w_qr"][:, 256 * r:256 * (r + 1)]
        wuk_c = g["w_ukv"][:, 512 * r:512 * (r + 1)]
        wuv_c = g["w_ukv"][:, C + 512 * r:C + 512 * (r + 1)]
        wuq_c = g["w_uq"][:, 512 * r:512 * (r + 1)]
        m = {
            "xT": xT_t,
            "ln1s": ln1s_t, "ln1b": ln1b_t, "ln2s": ln2s_t, "ln2b": ln2b_t,
            "wdown": wdown_t, "bdown": bdown_t,
            "wqr": np.ascontiguousarray(wqr_c.reshape(CC, P, 2 * P)),
            "bqr": np.ascontiguousarray(
                g["b_qr"][256 * r:256 * (r + 1)].reshape(2, P).T),
            "wkr": wkr_t, "bkr": bkr_t,
            "r2": r2,
            "cosq": cosq, "sinq": sinq, "cosk": cosk, "sink": sink,
            "wuk": np.ascontiguousarray(wuk_c.reshape(4, P, 4 * P)),
            "buk": np.ascontiguousarray(
                g["b_ukv"][512 * r:512 * (r + 1)].reshape(4, P).T),
            "wuv": np.ascontiguousarray(wuv_c.reshape(4, P, 4 * P)),
            "buv": np.ascontiguousarray(
                g["b_ukv"][C + 512 * r:C + 512 * (r + 1)].reshape(4, P).T),
            "wuq": np.ascontiguousarray(wuq_c.reshape(4, P, 4 * P)),
            "buq": np.ascontiguousarray(
                g["b_uq"][512 * r:512 * (r + 1)].reshape(4, P).T),
            "mask": mask, "ones_r": ones, "ones32": ones,
            "wo": wo_t, "bo": bo_t,
            "wff1": wff1_t, "bff1": bff1_t,
            "wff2": wff2_t, "bff2": bff2_t,
        }
        in_maps.append(m)
    return in_maps


def assemble_output(results):
    out = np.zeros((B, T, C), np.float32)
    for c in range(NCORES):
        b, r = divmod(c, 4)
        o = results[c]["outT"].reshape(C, NT)
        out[b, NT * r:NT * (r + 1), :] = o.T
    return out


def kernel(**inputs):
    from concourse import bass_utils
    nc = _CACHE.get("nc")
    if nc is None:
        nc = build_program(repeat=1)
        _CACHE["nc"] = nc
    in_maps = prepare_in_maps(inputs)
    res = bass_utils.run_bass_kernel_spmd(nc, in_maps,
                                          core_ids=list(range(NCORES)))
    return assemble_output(res.results)


# revision 7
# speedup vs baseline: 2.0139x; 2.0139x over previous
"""Trainium2 Bass kernel for nn_Block_9268539425531 (MLA transformer block).

Sharding: 2 batch groups x 4-way TP within each group of 4 cores.
Per core (b = core//4, r = core%4, heads H = [4r, 4r+4)):
  Phase A: ln1 + w_down on own token slice (512 tokens), AllGather h+dkv.
  Phase B: q/k/v/qR/kR projections for own 4 heads, all 2048 tokens,
           spilled to DRAM.
  Phase C: causal attention for own 4 heads (scoresT layout, matmul-based
           partition softmax reductions), AllGather oT.
  Phase D: w_o + residual + ln2 on own token slice.
  Phase E: FFN (full hidden dim, own token slice) + residual.
All matmuls in float32r (full-rate fp32, ~1.6e-4 rel rounding).
"""
import math
import numpy as np

B, T, C = 2, 2048, 2048
NH = 16
DK = 128
DHR = 64
LAT = 512
P = 128
NT = 512           # tokens per core
CC = C // P        # 16
NCORES = 8
SCALE = 1.0 / math.sqrt(DK)
NEG = -1.0e9
RG = [[0, 1, 2, 3], [4, 5, 6, 7]]

_CACHE = {}


# ---------------------------------------------------------------- program ---
def build_program(repeat=1, nocc=False):
    from contextlib import ExitStack
    from concourse import bass, bacc, tile, mybir

    dt = mybir.dt
    f32 = dt.float32
    f32r = dt.float32r
    AF = mybir.ActivationFunctionType
    OP = mybir.AluOpType

    nc = bacc.Bacc("TRN2", target_bir_lowering=False, debug=False,
                   num_devices=NCORES)

    def din(name, shape, dtype=f32r):
        return nc.dram_tensor(name, shape, dtype, kind="ExternalInput")

    xT_d = din("xT", [CC, P, NT], f32)
    ln1s_d = din("ln1s", [P, CC], f32)
    ln1b_d = din("ln1b", [P, CC], f32)
    ln2s_d = din("ln2s", [P, CC], f32)
    ln2b_d = din("ln2b", [P, CC], f32)
    wdown_d = din("wdown", [CC, P, 8 * P])
    bdown_d = din("bdown", [P, 8], f32)
    wqr_d = din("wqr", [CC, P, 2 * P])
    bqr_d = din("bqr", [P, 2], f32)
    wkr_d = din("wkr", [CC, P, P])
    bkr_d = din("bkr", [P, 1], f32)
    r2_d = din("r2", [P, P])
    cosq_d = din("cosq", [2, 4, P, NT], f32)
    sinq_d = din("sinq", [2, 4, P, NT], f32)
    cosk_d = din("cosk", [4, P, NT], f32)
    sink_d = din("sink", [4, P, NT], f32)
    wuk_d = din("wuk", [4, P, 4 * P])
    buk_d = din("buk", [P, 4], f32)
    wuv_d = din("wuv", [4, P, 4 * P])
    buv_d = din("buv", [P, 4], f32)
    wuq_d = din("wuq", [4, P, 4 * P])
    buq_d = din("buq", [P, 4], f32)
    mask_d = din("mask", [4, P, NT], f32)
    ones_r_d = din("ones_r", [P, P])
    ones32_d = din("ones32", [P, P], f32)
    wo_d = din("wo", [CC, 4, P, 4 * P])
    bo_d = din("bo", [P, CC], f32)
    wff1_d = din("wff1", [CC, 16, P, 4 * P])
    bff1_d = din("bff1", [P, 64], f32)
    wff2_d = din("wff2", [4, CC, P, CC * P])
    bff2_d = din("bff2", [P, CC], f32)
    outT_d = nc.dram_tensor("outT", [CC, P, NT], f32, kind="ExternalOutput")

    with tile.TileContext(nc) as tc, ExitStack() as ctx:
        pc = ctx.enter_context(tc.tile_pool(name="const", bufs=1))
        pdram = ctx.enter_context(tc.tile_pool(name="dram", bufs=1, space="DRAM"))

        # ---- small constants resident for the whole kernel (~3KB/part)
        ones_r = pc.tile([P, P], f32r)
        nc.sync.dma_start(ones_r[:], ones_r_d[:])
        ones32 = pc.tile([P, P], f32)
        nc.sync.dma_start(ones32[:], ones32_d[:])
        r2 = pc.tile([P, P], f32r)
        nc.sync.dma_start(r2[:], r2_d[:])
        ln1s = pc.tile([P, CC], f32)
        nc.sync.dma_start(ln1s[:], ln1s_d[:])
        ln1b = pc.tile([P, CC], f32)
        nc.sync.dma_start(ln1b[:], ln1b_d[:])
        ln2s = pc.tile([P, CC], f32)
        nc.sync.dma_start(ln2s[:], ln2s_d[:])
        ln2b = pc.tile([P, CC], f32)
        nc.sync.dma_start(ln2b[:], ln2b_d[:])
        bdown = pc.tile([P, 8], f32)
        nc.sync.dma_start(bdown[:], bdown_d[:])
        bqr = pc.tile([P, 2], f32)
        nc.sync.dma_start(bqr[:], bqr_d[:])
        bkr = pc.tile([P, 1], f32)
        nc.sync.dma_start(bkr[:], bkr_d[:])
        buk = pc.tile([P, 4], f32)
        nc.sync.dma_start(buk[:], buk_d[:])
        buv = pc.tile([P, 4], f32)
        nc.sync.dma_start(buv[:], buv_d[:])
        buq = pc.tile([P, 4], f32)
        nc.sync.dma_start(buq[:], buq_d[:])
        bo = pc.tile([P, CC], f32)
        nc.sync.dma_start(bo[:], bo_d[:])
        bff1 = pc.tile([P, 64], f32)
        nc.sync.dma_start(bff1[:], bff1_d[:])
        bff2 = pc.tile([P, CC], f32)
        nc.sync.dma_start(bff2[:], bff2_d[:])
        eps_t = pc.tile([P, 1], f32)
        nc.vector.memset(eps_t[:], 1e-6)

        agin1 = pdram.tile([3072, NT], f32r)          # h (2048) + dkv (1024)
        agout1 = pdram.tile([4 * 3072, NT], f32r)
        agin2 = pdram.tile([4 * P, T], f32r)          # own-heads oT
        agout2 = pdram.tile([16 * P, T], f32r)
        qR_sp = pdram.tile([2, P, T], f32r)           # projection spills
        kR_sp = pdram.tile([P, T], f32r)
        q_sp = pdram.tile([4, P, T], f32r)
        k_sp = pdram.tile([4, P, T], f32r)
        v_sp = pdram.tile([16, P, 4 * P], f32r)

        pid = nc.sync.partition_id()
        colo = (pid % 4) * NT

        def layer_norm(src_tiles, pstream, pstat, pool_ps, lns, lnb,
                       out_pool, out_name, rep):
            """src [16][P, NT] fp32 -> normalized f32r tiles (list)."""
            ps_mean = pool_ps.tile([P, NT], f32, name=f"lnpm{rep}{out_name}")
            ps_sq = pool_ps.tile([P, NT], f32, name=f"lnps{rep}{out_name}")
            for ci in range(CC):
                sq = pstream.tile([P, NT], f32r, name="lnsq", tag="lnsq")
                nc.scalar.square(sq[:], src_tiles[ci][:])
                nc.tensor.matmul(ps_mean[:], ones32[:], src_tiles[ci][:],
                                 start=(ci == 0), stop=(ci == CC - 1),
                                 skip_group_check=True)
                nc.tensor.matmul(ps_sq[:], ones_r[:], sq[:],
                                 start=(ci == 0), stop=(ci == CC - 1),
                                 skip_group_check=True)
            meanb = pstat.tile([P, NT], f32, name="lnmean", tag="lnmean")
            nc.vector.tensor_scalar_mul(meanb[:], ps_mean[:], 1.0 / C)
            m2 = pstat.tile([P, NT], f32, name="lnm2", tag="lnm2")
            nc.vector.tensor_mul(m2[:], meanb[:], meanb[:])
            var = pstat.tile([P, NT], f32, name="lnvar", tag="lnvar")
            nc.vector.scalar_tensor_tensor(var[:], ps_sq[:], 1.0 / C, m2[:],
                                           OP.mult, OP.subtract)
            std = pstat.tile([P, NT], f32, name="lnstd", tag="lnstd")
            nc.scalar.activation(std[:], var[:], AF.Sqrt, bias=eps_t[:])
            rstd = pstat.tile([P, NT], f32, name="lnrstd", tag="lnrstd")
            nc.vector.reciprocal(rstd[:], std[:])
            outs = []
            for ci in range(CC):
                t1 = pstream.tile([P, NT], f32, name="lnt1", tag="lnt1")
                nc.vector.tensor_sub(t1[:], src_tiles[ci][:], meanb[:])
                t2 = pstream.tile([P, NT], f32, name="lnt2", tag="lnt2")
                nc.vector.tensor_mul(t2[:], t1[:], rstd[:])
                h = out_pool.tile([P, NT], f32r, name=f"{out_name}{ci}")
                nc.vector.tensor_scalar(h[:], t2[:], lns[:, ci:ci + 1],
                                        lnb[:, ci:ci + 1], OP.mult, OP.add)
                outs.append(h)
            return outs

        for rep in range(repeat):
            # ------------------------------------------------ phase A ----
            with (tc.tile_pool(name=f"pxa{rep}", bufs=1) as pxa,
                  tc.tile_pool(name=f"pa{rep}", bufs=3) as pa,
                  tc.tile_pool(name=f"pas{rep}", bufs=1) as pas,
                  tc.tile_pool(name=f"pah{rep}", bufs=1) as pah,
                  tc.tile_pool(name=f"paw{rep}", bufs=20) as paw,
                  tc.tile_pool(name=f"paps{rep}", bufs=2, space="PSUM") as paps,
                  tc.tile_pool(name=f"past{rep}", bufs=1, space="PSUM") as pstat):
                xT = []
                for ci in range(CC):
                    t = pxa.tile([P, NT], f32, name=f"xTa{ci}")
                    nc.sync.dma_start(t[:], xT_d[ci])
                    xT.append(t)
                hts = layer_norm(xT, pa, pas, pstat, ln1s, ln1b, pah, "h", rep)
                for ci in range(CC):
                    nc.sync.dma_start(agin1[P * ci:P * (ci + 1), :], hts[ci][:])
                for mi in range(8):
                    ps = paps.tile([P, NT], f32, name="psdkv", tag="psdkv")
                    for ci in range(CC):
                        w = paw.tile([P, P], f32r, name="wdt", tag="wdt")
                        nc.sync.dma_start(w[:], wdown_d[ci, :, mi * P:(mi + 1) * P])
                        nc.tensor.matmul(ps[:], w[:], hts[ci][:],
                                         start=(ci == 0), stop=(ci == CC - 1))
                    dkv = pa.tile([P, NT], f32r, name="dkvt", tag="dkvt")
                    nc.vector.tensor_scalar_add(dkv[:], ps[:], bdown[:, mi:mi + 1])
                    nc.sync.dma_start(agin1[C + P * mi:C + P * (mi + 1), :], dkv[:])

            if nocc:
                nc.sync.dma_start(agout1[0:3072, :], agin1[:])
            else:
                nc.gpsimd.collective_compute(
                    "AllGather", mybir.AluOpType.bypass, replica_groups=RG,
                    ins=[agin1.opt()], outs=[agout1.opt()])

            # --------------------------------------------- phase B1 ----
            # qR, kR, q projections -> DRAM spills
            with (tc.tile_pool(name=f"pb1w{rep}", bufs=1) as pw,
                  tc.tile_pool(name=f"pb1s{rep}", bufs=18) as pstream,
                  tc.tile_pool(name=f"pb1c{rep}", bufs=6) as pcq,
                  tc.tile_pool(name=f"pb1t{rep}", bufs=3) as pt,
                  tc.tile_pool(name=f"pb1cs{rep}", bufs=3) as pcs,
                  tc.tile_pool(name=f"pb1ps{rep}", bufs=3, space="PSUM") as pps,
                  tc.tile_pool(name=f"pb1pr{rep}", bufs=2, space="PSUM") as ppsr):
                wqr_sb = []
                for ci in range(CC):
                    w = pw.tile([P, 2 * P], f32r, name=f"wqr{ci}")
                    nc.sync.dma_start(w[:], wqr_d[ci])
                    wqr_sb.append(w)
                wkr_sb = []
                for ci in range(CC):
                    w = pw.tile([P, P], f32r, name=f"wkr{ci}")
                    nc.sync.dma_start(w[:], wkr_d[ci])
                    wkr_sb.append(w)
                wuq_sb = []
                for lc in range(4):
                    w = pw.tile([P, 4 * P], f32r, name=f"wuq{lc}")
                    nc.sync.dma_start(w[:], wuq_d[lc])
                    wuq_sb.append(w)

                def rope(pre, cos_t, sin_t, dst_ap):
                    rot = ppsr.tile([P, NT], f32, name="psrot", tag="psrot")
                    nc.tensor.matmul(rot[:], r2[:], pre[:], start=True, stop=True)
                    tmp = pt.tile([P, NT], f32, name="rtmp", tag="rtmp")
                    nc.vector.tensor_mul(tmp[:], rot[:], sin_t[:])
                    tmp2 = pt.tile([P, NT], f32, name="rtmp2", tag="rtmp2")
                    nc.vector.tensor_mul(tmp2[:], pre[:], cos_t[:])
                    out = pt.tile([P, NT], f32r, name="rout", tag="rout")
                    nc.vector.tensor_add(out[:], tmp2[:], tmp[:])
                    nc.sync.dma_start(dst_ap, out[:])

                for nt in range(4):
                    nts = slice(nt * NT, (nt + 1) * NT)
                    ht = []
                    for ci in range(CC):
                        t = pstream.tile([P, NT], f32r, name="htb", tag="htb")
                        nc.sync.dma_start(
                            t[:], agout1[3072 * nt + P * ci:
                                         3072 * nt + P * (ci + 1), :])
                        ht.append(t)
                    cq = []
                    for lc in range(4):
                        t = pcq.tile([P, NT], f32r, name="cqb", tag="cqb")
                        nc.sync.dma_start(
                            t[:], agout1[3072 * nt + 2560 + P * lc:
                                         3072 * nt + 2560 + P * (lc + 1), :])
                        cq.append(t)
                    for mt in range(2):
                        ps = pps.tile([P, NT], f32, name="psqr", tag="psqr")
                        for ci in range(CC):
                            nc.tensor.matmul(
                                ps[:], wqr_sb[ci][:, mt * P:(mt + 1) * P],
                                ht[ci][:], start=(ci == 0), stop=(ci == CC - 1))
                        pre = pt.tile([P, NT], f32r, name="qrpre", tag="qrpre")
                        nc.scalar.activation(pre[:], ps[:], AF.Identity,
                                             bias=bqr[:, mt:mt + 1])
                        cos_t = pcs.tile([P, NT], f32, name="cosq", tag="cosq")
                        nc.sync.dma_start(cos_t[:], cosq_d[mt, nt])
                        sin_t = pcs.tile([P, NT], f32, name="sinq", tag="sinq")
                        nc.sync.dma_start(sin_t[:], sinq_d[mt, nt])
                        rope(pre, cos_t, sin_t, qR_sp[mt][:, nts])
                    # kR
                    ps = pps.tile([P, NT], f32, name="pskr", tag="psqr")
                    for ci in range(CC):
                        nc.tensor.matmul(ps[:], wkr_sb[ci][:], ht[ci][:],
                                         start=(ci == 0), stop=(ci == CC - 1))
                    pre = pt.tile([P, NT], f32r, name="krpre", tag="qrpre")
                    nc.scalar.activation(pre[:], ps[:], AF.Identity,
                                         bias=bkr[:, 0:1])
                    cos_t = pcs.tile([P, NT], f32, name="cosk", tag="cosq")
                    nc.sync.dma_start(cos_t[:], cosk_d[nt])
                    sin_t = pcs.tile([P, NT], f32, name="sink", tag="sinq")
                    nc.sync.dma_start(sin_t[:], sink_d[nt])
                    rope(pre, cos_t, sin_t, kR_sp[:, nts])
                    # q
                    for mt in range(4):
                        ps = pps.tile([P, NT], f32, name="psq", tag="psqr")
                        for lc in range(4):
                            nc.tensor.matmul(
                                ps[:], wuq_sb[lc][:, mt * P:(mt + 1) * P],
                                cq[lc][:], start=(lc == 0), stop=(lc == 3))
                        qo = pt.tile([P, NT], f32r, name="qout", tag="rout")
                        nc.vector.tensor_scalar_add(qo[:], ps[:],
                                                    buq[:, mt:mt + 1])
                        nc.sync.dma_start(q_sp[mt][:, nts], qo[:])

            # --------------------------------------------- phase B2 ----
            with (tc.tile_pool(name=f"pb2w{rep}", bufs=1) as pw2,
                  tc.tile_pool(name=f"pb2s{rep}", bufs=6) as pkv,
                  tc.tile_pool(name=f"pb2t{rep}", bufs=3) as pt2,
                  tc.tile_pool(name=f"pb2ps{rep}", bufs=3, space="PSUM") as pps2):
                wuk_sb = []
                wuv_sb = []
                for lc in range(4):
                    w = pw2.tile([P, 4 * P], f32r, name=f"wuk{lc}")
                    nc.sync.dma_start(w[:], wuk_d[lc])
                    wuk_sb.append(w)
                    w = pw2.tile([P, 4 * P], f32r, name=f"wuv{lc}")
                    nc.sync.dma_start(w[:], wuv_d[lc])
                    wuv_sb.append(w)
                for nt in range(4):
                    nts = slice(nt * NT, (nt + 1) * NT)
                    ckv = []
                    for lc in range(4):
                        t = pkv.tile([P, NT], f32r, name="ckvb", tag="ckvb")
                        nc.sync.dma_start(
                            t[:], agout1[3072 * nt + 2048 + P * lc:
                                         3072 * nt + 2048 + P * (lc + 1), :])
                        ckv.append(t)
                    for mt in range(4):
                        ps = pps2.tile([P, NT], f32, name="psk", tag="psk")
                        for lc in range(4):
                            nc.tensor.matmul(
                                ps[:], wuk_sb[lc][:, mt * P:(mt + 1) * P],
                                ckv[lc][:], start=(lc == 0), stop=(lc == 3))
                        ko = pt2.tile([P, NT], f32r, name="kout", tag="kout")
                        nc.vector.tensor_scalar_add(ko[:], ps[:],
                                                    buk[:, mt:mt + 1])
                        nc.sync.dma_start(k_sp[mt][:, nts], ko[:])
                    for tt in range(4):
                        ps = pps2.tile([P, 4 * P], f32, name="psv", tag="psk")
                        for lc in range(4):
                            nc.tensor.matmul(
                                ps[:], ckv[lc][:, tt * P:(tt + 1) * P],
                                wuv_sb[lc][:], start=(lc == 0), stop=(lc == 3))
                        vo = pt2.tile([P, 4 * P], f32r, name="vout", tag="vout")
                        nc.vector.tensor_copy(vo[:], ps[:])
                        nc.sync.dma_start(v_sp[4 * nt + tt], vo[:])

            # ---------------------------------------------- phase C ----
            with (tc.tile_pool(name=f"pch{rep}", bufs=2) as phd,
                  tc.tile_pool(name=f"pcm{rep}", bufs=1) as pcm,
                  tc.tile_pool(name=f"pce{rep}", bufs=4) as pex,
                  tc.tile_pool(name=f"pco{rep}", bufs=3) as pot,
                  tc.tile_pool(name=f"pcps{rep}", bufs=3, space="PSUM") as pcsc,
                  tc.tile_pool(name=f"pcpo{rep}", bufs=2, space="PSUM") as pcso,
                  tc.tile_pool(name=f"pcpm{rep}", bufs=2, space="PSUM") as pcss):
                masks = []
                for j in range(4):
                    m_ = pcm.tile([P, NT], f32, name=f"mask{j}")
                    nc.sync.dma_start(m_[:], mask_d[j])
                    masks.append(m_)
                for h in range(4):
                    kTh = phd.tile([P, T], f32r, name="kTh", tag="kTh")
                    nc.sync.dma_start(kTh[:], k_sp[h])
                    qTh = phd.tile([P, T], f32r, name="qTh", tag="qTh")
                    nc.sync.dma_start(qTh[:], q_sp[h])
                    qRh = phd.tile([DHR, T], f32r, name="qRh", tag="qRh")
                    nc.sync.dma_start(
                        qRh[:], qR_sp[h // 2][DHR * (h % 2):DHR * (h % 2) + DHR, :])
                    kRh = phd.tile([DHR, T], f32r, name="kRh", tag="kRh")
                    nc.sync.dma_start(kRh[:], kR_sp[0:DHR, :])
                    vh = phd.tile([P, 16, P], f32r, name="vh", tag="vh")
                    for tt in range(16):
                        nc.sync.dma_start(vh[:, tt, :],
                                          v_sp[tt][:, h * P:(h + 1) * P])
                    for qi in range(4):
                        qs = slice(qi * NT, (qi + 1) * NT)
                        pso = pcso.tile([P, NT], f32, name="pso", tag="pso")
                        pss = pcss.tile([P, NT], f32, name="pss", tag="pss")
                        nki = 4 * qi + 4
                        for ki in range(nki):
                            ks = slice(ki * P, (ki + 1) * P)
                            psc = pcsc.tile([P, NT], f32, name="psc", tag="psc")
                            nc.tensor.matmul(psc[:], kTh[:, ks], qTh[:, qs],
                                             start=True, stop=False)
                            nc.tensor.matmul(psc[:], kRh[:, ks], qRh[:, qs],
                                             start=False, stop=True)
                            if ki >= 4 * qi:
                                nc.vector.tensor_add(psc[:], psc[:],
                                                     masks[ki - 4 * qi][:])
                            ex = pex.tile([P, NT], f32r, name="ex", tag="ex")
                            nc.scalar.activation(ex[:], psc[:], AF.Exp,
                                                 scale=SCALE)
                            nc.tensor.matmul(pso[:], vh[:, ki, :], ex[:],
                                             start=(ki == 0),
                                             stop=(ki == nki - 1))
                            nc.tensor.matmul(pss[:], ones_r[:], ex[:],
                                             start=(ki == 0),
                                             stop=(ki == nki - 1))
                        rec = pot.tile([P, NT], f32, name="rec", tag="rec")
                        nc.vector.reciprocal(rec[:], pss[:])
                        ot = pot.tile([P, NT], f32, name="ot", tag="ot")
                        nc.vector.tensor_mul(ot[:], pso[:], rec[:])
                        otb = pot.tile([P, NT], f32r, name="otb", tag="otb")
                        nc.vector.tensor_scalar_add(otb[:], ot[:],
                                                    buv[:, h:h + 1])
                        nc.sync.dma_start(agin2[h * P:(h + 1) * P, qs], otb[:])

            if nocc:
                nc.sync.dma_start(agout2[0:4 * P, :], agin2[:])
            else:
                nc.gpsimd.collective_compute(
                    "AllGather", mybir.AluOpType.bypass, replica_groups=RG,
                    ins=[agin2.opt()], outs=[agout2.opt()])

            # ------------------------------------------------ phase D ----
            with tc.tile_pool(name=f"pde{rep}", bufs=1) as pper:
                with (tc.tile_pool(name=f"pxd{rep}", bufs=1) as pxd,
                      tc.tile_pool(name=f"pdo{rep}", bufs=1) as pdo,
                      tc.tile_pool(name=f"pdw{rep}", bufs=17) as pdw,
                      tc.tile_pool(name=f"pdt{rep}", bufs=3) as pdt,
                      tc.tile_pool(name=f"pds{rep}", bufs=1) as pds,
                      tc.tile_pool(name=f"pdps{rep}", bufs=3, space="PSUM") as pdps,
                      tc.tile_pool(name=f"pdst{rep}", bufs=1, space="PSUM") as pdst):
                    xT2 = []
                    for ci in range(CC):
                        t = pxd.tile([P, NT], f32, name=f"xTd{ci}")
                        nc.sync.dma_start(t[:], xT_d[ci])
                        xT2.append(t)
                    otsl = []
                    for oi in range(16):
                        t = pdo.tile([P, NT], f32r, name=f"otsl{oi}")
                        nc.sync.dma_start(
                            t[:], agout2[oi * P:(oi + 1) * P, bass.ds(colo, NT)])
                        otsl.append(t)
                    xmid = []
                    for mig in range(4):
                        wots = []
                        for ki in range(16):
                            w = pdw.tile([P, 4 * P], f32r, name="wot", tag="wot")
                            nc.sync.dma_start(w[:], wo_d[ki, mig])
                            wots.append(w)
                        for ml in range(4):
                            mi = mig * 4 + ml
                            ps = pdps.tile([P, NT], f32, name="pswo", tag="pswo")
                            for ki in range(16):
                                nc.tensor.matmul(
                                    ps[:], wots[ki][:, ml * P:(ml + 1) * P],
                                    otsl[ki][:], start=(ki == 0), stop=(ki == 15))
                            xm = pper.tile([P, NT], f32, name=f"xmid{mi}")
                            nc.vector.scalar_tensor_tensor(
                                xm[:], ps[:], bo[:, mi:mi + 1], xT2[mi][:],
                                OP.add, OP.add)
                            xmid.append(xm)
                    h2 = layer_norm(xmid, pdt, pds, pdst, ln2s, ln2b, pper,
                                    "h2_", rep)

                # -------------------------------------------- phase E ----
                with (tc.tile_pool(name=f"pew{rep}", bufs=17) as pew,
                      tc.tile_pool(name=f"pew2{rep}", bufs=2) as pew2,
                      tc.tile_pool(name=f"peg{rep}", bufs=17) as peg,
                      tc.tile_pool(name=f"pea{rep}", bufs=1) as pea,
                      tc.tile_pool(name=f"pet{rep}", bufs=3) as pet,
                      tc.tile_pool(name=f"peps{rep}", bufs=3, space="PSUM") as peps,
                      tc.tile_pool(name=f"pep2{rep}", bufs=2, space="PSUM") as pep2):
                    accs = [pea.tile([P, NT], f32, name=f"ffacc{mi}")
                            for mi in range(16)]
                    for hb in range(4):
                        gts = []
                        for mtg in range(4):
                            mtg_g = hb * 4 + mtg
                            wts = []
                            for ci in range(CC):
                                w = pew.tile([P, 4 * P], f32r, name="wf1",
                                             tag="wf1")
                                nc.sync.dma_start(w[:], wff1_d[ci, mtg_g])
                                wts.append(w)
                            for ml in range(4):
                                mt = mtg_g * 4 + ml
                                ps = peps.tile([P, NT], f32, name="psf1",
                                               tag="psf1")
                                for ci in range(CC):
                                    nc.tensor.matmul(
                                        ps[:], wts[ci][:, ml * P:(ml + 1) * P],
                                        h2[ci][:],
                                        start=(ci == 0), stop=(ci == CC - 1))
                                gt = peg.tile([P, NT], f32r, name="gt", tag="gt")
                                nc.scalar.activation(gt[:], ps[:],
                                                     AF.Gelu_apprx_tanh,
                                                     bias=bff1[:, mt:mt + 1])
                                gts.append(gt)
                        for mi in range(16):
                            w2 = pew2.tile([P, CC * P], f32r, name="wf2",
                                           tag="wf2")
                            nc.sync.dma_start(w2[:], wff2_d[hb, mi])
                            ps2 = pep2.tile([P, NT], f32, name="psf2",
                                            tag="psf2")
                            for hl in range(16):
                                nc.tensor.matmul(
                                    ps2[:], w2[:, hl * P:(hl + 1) * P],
                                    gts[hl][:],
                                    start=(hl == 0), stop=(hl == 15))
                            if hb == 0:
                                nc.vector.tensor_copy(accs[mi][:], ps2[:])
                            else:
                                nc.vector.tensor_add(accs[mi][:], accs[mi][:],
                                                     ps2[:])
                    for mi in range(CC):
                        ob = pet.tile([P, NT], f32, name="outb", tag="outb")
                        nc.vector.scalar_tensor_tensor(
                            ob[:], accs[mi][:], bff2[:, mi:mi + 1],
                            xmid[mi][:], OP.add, OP.add)
                        nc.sync.dma_start(outT_d[mi], ob[:])

    nc.compile()
    return nc


# ------------------------------------------------------------------ host ---
def _rope_tables(r):
    """cos/sin tiles for core rank r (heads 4r..4r+3)."""
    t = np.arange(T, dtype=np.float64) + 1.0
    l = np.arange(DHR)
    cosq = np.zeros((2, P, T), np.float64)
    sinq = np.zeros((2, P, T), np.float64)
    for mt in range(2):
        for hl in range(2):
            h = 4 * r + 2 * mt + hl
            theta = 10000.0 ** (-2.0 * (32 * h + l // 2) / 1024.0)
            ang = t[None, :] * theta[:, None]            # [64, T]
            cosq[mt, 64 * hl:64 * hl + 64] = np.cos(ang)
            sinq[mt, 64 * hl:64 * hl + 64] = np.sin(ang)
    thk = 10000.0 ** (-2.0 * (l // 2) / 64.0)
    angk = t[None, :] * thk[:, None]
    cosk = np.concatenate([np.cos(angk)] * 2, axis=0)     # [128, T]
    sink = np.concatenate([np.sin(angk)] * 2, axis=0)
    cosq = cosq.reshape(2, P, 4, NT).transpose(0, 2, 1, 3)
    sinq = sinq.reshape(2, P, 4, NT).transpose(0, 2, 1, 3)
    cosk = cosk.reshape(P, 4, NT).transpose(1, 0, 2)
    sink = sink.reshape(P, 4, NT).transpose(1, 0, 2)
    f = np.float32
    return (np.ascontiguousarray(cosq, f), np.ascontiguousarray(sinq, f),
            np.ascontiguousarray(cosk, f), np.ascontiguousarray(sink, f))


def _shared_consts():
    r2 = np.zeros((P, P), np.float32)
    for i in range(64):
        r2[2 * i + 1, 2 * i] = -1.0
        r2[2 * i, 2 * i + 1] = 1.0
    mask = np.zeros((4, P, NT), np.float32)
    kl = np.arange(P)[:, None]
    ql = np.arange(NT)[None, :]
    for j in range(4):
        mask[j] = np.where(P * j + kl > ql, NEG, 0.0)
    ones = np.ones((P, P), np.float32)
    return r2, mask, ones


def prepare_in_maps(inputs):
    f = np.float32
    g = {k: np.asarray(v, f) for k, v in inputs.items()}
    x = g["x"]
    r2, mask, ones = _shared_consts()

    wdown_t = np.ascontiguousarray(g["w_down"].reshape(CC, P, 8 * P))
    bdown_t = np.ascontiguousarray(g["b_down"].reshape(8, P).T)
    wkr2 = np.concatenate([g["w_kr"], g["w_kr"]], axis=1)  # [C, 128]
    wkr_t = np.ascontiguousarray(wkr2.reshape(CC, P, P))
    bkr_t = np.ascontiguousarray(
        np.concatenate([g["b_kr"], g["b_kr"]]).reshape(P, 1))
    wo_t = np.ascontiguousarray(
        g["w_o"].reshape(CC, P, 4, 4 * P).transpose(0, 2, 1, 3))
    bo_t = np.ascontiguousarray(g["b_o"].reshape(CC, P).T)
    wff1_t = np.ascontiguousarray(
        g["w_ff1"].reshape(CC, P, 16, 4 * P).transpose(0, 2, 1, 3))
    bff1_t = np.ascontiguousarray(g["b_ff1"].reshape(64, P).T)
    wff2_t = np.ascontiguousarray(
        g["w_ff2"].reshape(4, CC, P, CC, P).transpose(0, 3, 2, 1, 4)
        .reshape(4, CC, P, CC * P))
    bff2_t = np.ascontiguousarray(g["b_ff2"].reshape(CC, P).T)
    ln1s_t = np.ascontiguousarray(g["ln1_scale"].reshape(CC, P).T)
    ln1b_t = np.ascontiguousarray(g["ln1_bias"].reshape(CC, P).T)
    ln2s_t = np.ascontiguousarray(g["ln2_scale"].reshape(CC, P).T)
    ln2b_t = np.ascontiguousarray(g["ln2_bias"].reshape(CC, P).T)

    in_maps = []
    for c in range(NCORES):
        b, r = divmod(c, 4)
        cosq, sinq, cosk, sink = _rope_tables(r)
        xs = x[b, NT * r:NT * (r + 1), :].T                      # [C, NT]
        xT_t = np.ascontiguousarray(xs.reshape(CC, P, NT))
        wqr_c = g["w_qr"][:, 256 * r:256 * (r + 1)]
        wuk_c = g["w_ukv"][:, 512 * r:512 * (r + 1)]
        wuv_c = g["w_ukv"][:, C + 512 * r:C + 512 * (r + 1)]
        wuq_c = g["w_uq"][:, 512 * r:512 * (r + 1)]
        m = {
            "xT": xT_t,
            "ln1s": ln1s_t, "ln1b": ln1b_t, "ln2s": ln2s_t, "ln2b": ln2b_t,
            "wdown": wdown_t, "bdown": bdown_t,
            "wqr": np.ascontiguousarray(wqr_c.reshape(CC, P, 2 * P)),
            "bqr": np.ascontiguousarray(
                g["b_qr"][256 * r:256 * (r + 1)].reshape(2, P).T),
            "wkr": wkr_t, "bkr": bkr_t,
            "r2": r2,
            "cosq": cosq, "sinq": sinq, "cosk": cosk, "sink": sink,
            "wuk": np.ascontiguousarray(wuk_c.reshape(4, P, 4 * P)),
            "buk": np.ascontiguousarray(
                g["b_ukv"][512 * r:512 * (r + 1)].reshape(4, P).T),
            "wuv": np.ascontiguousarray(wuv_c.reshape(4, P, 4 * P)),
            "buv": np.ascontiguousarray(
                g["b_ukv"][C + 512 * r:C + 512 * (r + 1)].reshape(4, P).T),
            "wuq": np.ascontiguousarray(wuq_c.reshape(4, P, 4 * P)),
            "buq": np.ascontiguousarray(
                g["b_uq"][512 * r:512 * (r + 1)].reshape(4, P).T),
            "mask": mask, "ones_r": ones, "ones32": ones,
            "wo": wo_t, "bo": bo_t,
            "wff1": wff1_t, "bff1": bff1_t,
            "wff2": wff2_t, "bff2": bff2_t,
        }
        in_maps.append(m)
    return in_maps


def assemble_output(results):
    out = np.zeros((B, T, C), np.float32)
    for c in range(NCORES):
        b, r = divmod(c, 4)
        o = results[c]["outT"].reshape(C, NT)
        out[b, NT * r:NT * (r + 1), :] = o.T
    return out


def kernel(**inputs):
    from concourse import bass_utils
    nc = _CACHE.get("nc")
    if nc is None:
        nc = build_program(repeat=1)
        _CACHE["nc"] = nc
    in_maps = prepare_in_maps(inputs)
    res = bass_utils.run_bass_kernel_spmd(nc, in_maps,
                                          core_ids=list(range(NCORES)))
    return assemble_output(res.results)
